# revision 1
# baseline (speedup 1.0000x reference)
"""Trainium2 Bass kernel for BeliefGNN message passing (8 NeuronCores), v3.

Strategy: shard the 3.2M directed messages by DESTINATION node range
(core c owns nodes [c*12544, (c+1)*12544)).  Host sorts directed edges by
destination and groups them into 128-node destination blocks (messages
within a block sub-grouped by source bank for the int16 gather).

The MLP's first layer is applied per-NODE on the host as a basis change:
  u[n] = nodes[n] @ W1b   (gathered per message source, bf16 256B rows)
  v[n] = nodes[n] @ W1a + b1   (loaded per destination block, bf16)
Per block the device then:
  - gathers u rows for message sources (4 banked calls on 4 SWDGE queues);
    the gathered tile IS the source z-contribution in [msg, H] layout
  - expands v to messages via a one-hot matmul against the block's 128 rows
  - z = s2T^T @ v + u_gathered  (PSUM + DVE add), ReLU
  - segment-sums messages into the block via the one-hot scatter matmul;
    the W2 projection is applied after the segment sum (commutes)
The per-group work is software-pipelined in 3 stages (prep / z+relu /
scatter) so the in-order PE queue never waits on ACT/DVE results.
No cross-core collective is needed: each core emits its own output slice.
"""

import numpy as np

N = 100000
D = 64
H = 128
E = 1600000
NCORES = 8
P = 128
B = 98                 # 128-node blocks per core
RANGE = B * P          # 12544 nodes owned per core
BANK = 25000           # source-bank width (int16 gather offset limit 32767)
NBANK = 4
BANKROWS = 32768       # uniform in_ap height per bank
NODESG = 3 * BANK + BANKROWS  # padded global table height
FORCE_SUBTPB = None
SINGLE_PACKET = False
REPS = 1               # timing aid: repeat the whole block loop on-device
NQ = 4                 # SWDGE queues for gather descriptor generation


def _build_program(subtpb, mode="full", single_packet=None, reps=None):
    """Build the Bass/Tile program. subtpb[k] = tiles per source bank."""
    import concourse.bass as bass
    import concourse.bacc as bacc
    import concourse.mybir as mybir
    import concourse.tile as tile

    sp = SINGLE_PACKET if single_packet is None else single_packet
    nreps = REPS if reps is None else reps
    do_gather = mode in ("full", "gatheronly")
    do_compute = mode in ("full", "nogather")

    f32 = mybir.dt.float32
    bf16 = mybir.dt.bfloat16
    i16 = mybir.dt.int16
    TILES = sum(subtpb)
    G4 = TILES // 4
    assert TILES % 4 == 0
    offs = np.concatenate([[0], np.cumsum(subtpb)]).astype(int)

    nc = bacc.Bacc(None, target_bir_lowering=False, num_swdge_queues=NQ)

    utabd = nc.dram_tensor("utab", [NODESG, P], bf16, kind="ExternalInput")
    vtabd = nc.dram_tensor("vtab", [RANGE, H], bf16, kind="ExternalInput")
    srcidx = nc.dram_tensor("srcidx16", [B // 2, P, 2 * TILES * 8], i16, kind="ExternalInput")
    dstlc = nc.dram_tensor("dstlc", [B, P, TILES], bf16, kind="ExternalInput")
    dstlcT = nc.dram_tensor("dstlcT", [B, 1, TILES * P], bf16, kind="ExternalInput")
    based = nc.dram_tensor("base", [RANGE, D], f32, kind="ExternalInput")
    w2d = nc.dram_tensor("w2b", [H, D], bf16, kind="ExternalInput")
    iotard = nc.dram_tensor("iotar4", [P, 4 * P], bf16, kind="ExternalInput")
    iotapd = nc.dram_tensor("iotap", [P, 1], f32, kind="ExternalInput")
    onesd = nc.dram_tensor("ones1", [1, P], bf16, kind="ExternalInput")
    outd = nc.dram_tensor("out", [RANGE, D], f32, kind="ExternalOutput")

    with tile.TileContext(nc) as tc:
        with (
            tc.tile_pool(name="const", bufs=1) as cp,
            tc.tile_pool(name="blk", bufs=3) as bp,
            tc.tile_pool(name="gs", bufs=3) as gp,
            tc.tile_pool(name="work", bufs=4) as wp,
            tc.tile_pool(name="ps_bc", bufs=2, space="PSUM") as ps_bc,
            tc.tile_pool(name="ps_z", bufs=2, space="PSUM") as ps_z,
            tc.tile_pool(name="ps_rt", bufs=2, space="PSUM") as ps_rt,
            tc.tile_pool(name="ps_d", bufs=1, space="PSUM") as ps_d,
        ):
            w2b = cp.tile([H, D], bf16)
            nc.sync.dma_start(out=w2b[:], in_=w2d[:])
            iotar4 = cp.tile([P, 4 * P], bf16)
            nc.sync.dma_start(out=iotar4[:], in_=iotard[:])
            iotap = cp.tile([P, 1], f32)
            nc.sync.dma_start(out=iotap[:], in_=iotapd[:])
            ones1 = cp.tile([1, P], bf16)
            nc.sync.dma_start(out=ones1[:], in_=onesd[:])

            import contextlib
            rep_ctx = tc.For_i(0, nreps, 1) if nreps > 1 else contextlib.nullcontext()
            with rep_ctx:
              for sb in range(B // 2):
                  idx_s = bp.tile([P, 2 * TILES * 8], i16, tag="idxs")
                  nc.sync.dma_start(out=idx_s[:], in_=srcidx[sb])
                  gus = gp.tile([P, 2 * TILES * P], bf16, tag="gus")
                  for k in range(NBANK if do_gather else 0):
                      if subtpb[k] == 0:
                          continue
                      nc.gpsimd.dma_gather(
                          out_ap=gus[:, 2 * offs[k] * P : 2 * offs[k + 1] * P].rearrange(
                              "p (t d) -> p t d", d=P
                          ),
                          in_ap=utabd[k * BANK : k * BANK + BANKROWS, :],
                          idxs_ap=idx_s[:, 2 * offs[k] * 8 : 2 * offs[k + 1] * 8],
                          num_idxs=2 * subtpb[k] * P,
                          num_idxs_reg=2 * subtpb[k] * P,
                          elem_size=P,
                          single_packet=sp,
                          queue_num=k % NQ,
                      )
                  if not do_gather:
                      nc.vector.memset(gus[:, 0 : 2 * TILES * P], 0.0)

                  for j in range(2):
                    b = 2 * sb + j

                    def gcol(t):
                        k = int(np.searchsorted(offs, t, side="right")) - 1
                        return 2 * offs[k] * P + j * subtpb[k] * P + (t - offs[k]) * P

                    def bank_runs(g):
                        res, t0, end = [], 4 * g, 4 * g + 4
                        while t0 < end:
                            k = int(np.searchsorted(offs, t0, side="right")) - 1
                            t1 = min(end, int(offs[k + 1]))
                            res.append((t0, t1))
                            t0 = t1
                        return res

                    dlc = bp.tile([P, TILES], bf16, tag="dlc")
                    nc.sync.dma_start(out=dlc[:], in_=dstlc[b])
                    dlcT = bp.tile([1, TILES * P], bf16, tag="dlcT")
                    nc.sync.dma_start(out=dlcT[:], in_=dstlcT[b])
                    vbs = bp.tile([P, H], bf16, tag="vbs")
                    nc.sync.dma_start(out=vbs[:], in_=vtabd[b * P : (b + 1) * P, :])
                    bst = bp.tile([P, D], f32, tag="base")
                    nc.sync.dma_start(out=bst[:], in_=based[b * P : (b + 1) * P, :])

                    if do_compute:
                        rt = ps_rt.tile([H, P], f32, tag="rt")
                        s2_l, s2T_l, rl_l = {}, {}, {}
                        for st in range(G4 + 2):
                            if st < G4:
                                g = st
                                s2 = wp.tile([P, 4 * P], bf16, tag="s2")
                                nc.vector.tensor_tensor(
                                    out=s2[:].rearrange("p (t j) -> p t j", t=4),
                                    in0=dlc[:, 4 * g : 4 * g + 4].to_broadcast([P, 4, P]),
                                    in1=iotar4[:, :].rearrange("p (t j) -> p t j", t=4),
                                    op=mybir.AluOpType.is_equal,
                                )
                                bc = ps_bc.tile([P, 4 * P], f32, tag="bc")
                                nc.tensor.matmul(
                                    out=bc[:],
                                    lhsT=ones1[:],
                                    rhs=dlcT[:, 4 * g * P : 4 * (g + 1) * P],
                                    start=True,
                                    stop=True,
                                )
                                s2T = wp.tile([P, 4 * P], bf16, tag="s2T")
                                nc.vector.tensor_tensor(
                                    out=s2T[:],
                                    in0=bc[:],
                                    in1=iotap[:, :].to_broadcast([P, 4 * P]),
                                    op=mybir.AluOpType.is_equal,
                                )
                                s2_l[g], s2T_l[g] = s2, s2T
                            if 1 <= st <= G4:
                                g = st - 1
                                z4 = ps_z.tile([P, 4 * H], f32, tag="z4")
                                for t in range(4):
                                    nc.tensor.matmul(
                                        out=z4[:, t * H : (t + 1) * H],
                                        lhsT=s2T_l[g][:, t * P : (t + 1) * P],
                                        rhs=vbs[:],
                                        start=True,
                                        stop=True,
                                    )
                                zb = wp.tile([P, 4 * H], bf16, tag="zb")
                                for (t0, t1) in bank_runs(g):
                                    nc.vector.tensor_tensor(
                                        out=zb[:, (t0 - 4 * g) * H : (t1 - 4 * g) * H],
                                        in0=z4[:, (t0 - 4 * g) * H : (t1 - 4 * g) * H],
                                        in1=gus[:, gcol(t0) : gcol(t0) + (t1 - t0) * P],
                                        op=mybir.AluOpType.add,
                                    )
                                rl = wp.tile([P, 4 * H], bf16, tag="rl")
                                nc.scalar.activation(
                                    out=rl[:],
                                    in_=zb[:],
                                    func=mybir.ActivationFunctionType.Relu,
                                )
                                rl_l[g] = rl
                            if st >= 2:
                                g = st - 2
                                for t in range(4):
                                    nc.tensor.matmul(
                                        out=rt[:],
                                        lhsT=rl_l[g][:, t * H : (t + 1) * H],
                                        rhs=s2_l[g][:, t * P : (t + 1) * P],
                                        start=(g == 0 and t == 0),
                                        stop=(g == G4 - 1 and t == 3),
                                    )
                        rts = wp.tile([H, P], bf16, tag="rts")
                        nc.vector.tensor_copy(out=rts[:], in_=rt[:])
                        delta = ps_d.tile([P, D], f32, tag="delta")
                        nc.tensor.matmul(
                            out=delta[:], lhsT=rts[:], rhs=w2b[:], start=True, stop=True
                        )
                    osb = bp.tile([P, D], f32, tag="osb")
                    if do_compute:
                        nc.vector.tensor_add(out=osb[:], in0=bst[:], in1=delta[:])
                    elif do_gather:
                        nc.vector.tensor_add(out=osb[:], in0=bst[:], in1=gus[:, 0:D])
                    else:
                        nc.vector.tensor_copy(out=osb[:], in_=bst[:])
                    nc.sync.dma_start(out=outd[b * P : (b + 1) * P, :], in_=osb[:])

    nc.compile()
    return nc


def _wrap16(a):
    """Pack a flat int array -> [128, len/16] int16 in the dma_gather idx
    layout (idx q at [q%16, q//16], replicated across the 8 gpsimd cores)."""
    a = np.asarray(a, np.int16).reshape(-1, 16).T  # [16, n/16]
    return np.tile(a, (8, 1))  # [128, n/16]


def _prep(nodes, edges, W1, b1, W2, b2):
    """Host-side: sort directed messages by destination, shard by dest range,
    group by 128-node dest block and source bank. Returns (in_maps, subtpb)."""
    import ml_dtypes

    def tobf(x):
        return np.ascontiguousarray(x).astype(ml_dtypes.bfloat16)

    nodes = np.ascontiguousarray(nodes, dtype=np.float32)
    edges = np.asarray(edges)
    dst = np.concatenate([edges[:, 0], edges[:, 1]]).astype(np.int64)
    src = np.concatenate([edges[:, 1], edges[:, 0]]).astype(np.int64)
    # sort by (dest block-of-128, source bank): dest-block-major key
    sbank_all = np.minimum(src // BANK, NBANK - 1)
    key = ((dst >> 7) << 2) | sbank_all
    order = np.argsort(key, kind="stable")
    dst = dst[order]
    src = src[order]
    sbank = sbank_all[order]

    bounds = np.searchsorted(dst, np.arange(NCORES + 1) * RANGE)
    per_core = []
    cnts = np.zeros((NCORES, B, NBANK), np.int64)
    for c in range(NCORES):
        dl = dst[bounds[c] : bounds[c + 1]] - c * RANGE
        sl = src[bounds[c] : bounds[c + 1]]
        sb = sbank[bounds[c] : bounds[c + 1]]
        blk = dl >> 7
        np.add.at(cnts[c], (blk, sb), 1)
        per_core.append((dl, sl, sb, blk))

    maxk = cnts.max(axis=(0, 1))            # per-bank max count
    subtpb = [int(-(-m // P)) for m in maxk]
    subtpb = [max(s, 1) for s in subtpb]
    while sum(subtpb) % 4:
        subtpb[0] += 1
    if FORCE_SUBTPB is not None:
        subtpb = list(FORCE_SUBTPB)
    TILES = sum(subtpb)
    offs = np.concatenate([[0], np.cumsum(subtpb)]).astype(np.int64)

    nodes_my = np.zeros((max(NCORES * RANGE, N) + RANGE, D), np.float32)
    nodes_my[:N] = nodes

    iotar4 = np.ascontiguousarray(
        np.broadcast_to(
            np.tile(np.arange(P, dtype=np.float32), 4)[None, :], (P, 4 * P)
        )
    )
    iotap = np.arange(P, dtype=np.float32).reshape(P, 1)
    ones1 = np.ones((1, P), np.float32)
    W1 = np.ascontiguousarray(W1, dtype=np.float32)
    W2 = np.ascontiguousarray(W2, dtype=np.float32)

    # host basis change: u = h @ W1b (gathered), v = h @ W1a + b1 (per block)
    u_all = nodes @ W1[D:]                       # [N, H] f32
    utab = np.zeros((NODESG, P), np.float32)
    utab[:N] = u_all
    utab_bf = tobf(utab)

    in_maps = []
    for c in range(NCORES):
        dl, sl, sb, blk = per_core[c]
        # slot of each message: position within its (block, bank) group
        grp = blk * NBANK + sb
        gstarts = np.concatenate(
            [[0], np.cumsum(np.bincount(grp, minlength=B * NBANK))]
        )[:-1]
        m = np.arange(len(dl)) - gstarts[grp]
        slot = (offs[sb] * P + m).astype(np.int64)  # slot within block slotspace
        tt = slot // P
        pp = slot % P

        src_flat = np.zeros((B, TILES * P), np.int64)
        dlc_arr = np.full((B, P, TILES), -1.0, np.float32)
        dlcT_arr = np.full((B, 1, TILES * P), -1.0, np.float32)
        src_flat[blk, slot] = sl - sb * BANK
        dlc_arr[blk, pp, tt] = (dl & 127).astype(np.float32)
        dlcT_arr[blk, 0, slot] = (dl & 127).astype(np.float32)

        srcidx16 = np.zeros((B // 2, P, 2 * TILES * 8), np.int16)
        for s in range(B // 2):
            parts = []
            for k in range(NBANK):
                for j in range(2):
                    parts.append(
                        _wrap16(
                            src_flat[2 * s + j, offs[k] * P : offs[k + 1] * P]
                        )
                    )
            srcidx16[s] = np.concatenate(parts, axis=1)

        deg = np.bincount(dl, minlength=RANGE).astype(np.float32)
        mynodes = np.ascontiguousarray(nodes_my[c * RANGE : (c + 1) * RANGE])
        base = mynodes + deg[:, None] * b2[None, :].astype(np.float32)
        vtab = mynodes @ W1[:D] + b1.astype(np.float32)[None, :]

        in_maps.append(
            {
                "utab": utab_bf,
                "vtab": tobf(vtab),
                "srcidx16": srcidx16,
                "dstlc": tobf(dlc_arr),
                "dstlcT": tobf(dlcT_arr),
                "base": np.ascontiguousarray(base),
                "w2b": tobf(W2),
                "iotar4": tobf(iotar4),
                "iotap": iotap,
                "ones1": tobf(ones1),
            }
        )
    return in_maps, subtpb


def kernel(nodes, edges, W1, b1, W2, b2):
    from concourse.bass_utils import run_bass_kernel_spmd

    in_maps, subtpb = _prep(nodes, edges, W1, b1, W2, b2)
    nc = _build_program(subtpb)
    res = run_bass_kernel_spmd(nc, in_maps, list(range(NCORES)))
    outs = [np.asarray(r["out"]) for r in res.results]
    return np.concatenate(outs, axis=0)[:N]



# revision 19
# speedup vs baseline: 25.8440x; 25.8440x over previous
"""Trainium2 Bass kernel for BeliefGNN message passing (8 NeuronCores), v6.

Strategy: shard the 3.2M directed messages by DESTINATION node range
(core c owns nodes [c*12544, (c+1)*12544)).  Host sorts directed edges by
destination and groups them into 128-node destination blocks (messages
within a block sub-grouped by source bank for the int16 gather).

The MLP's first layer is applied per-NODE on the host as a basis change:
  u[n] = nodes[n] @ W1b   (gathered per message source, bf16 256B rows)
  v[n] = nodes[n] @ W1a + b1   (loaded per destination block, bf16)
Per block the device then:
  - gathers u rows for message sources (4 banked calls on 4 SWDGE queues);
    the gathered tile IS the source z-contribution in [msg, H] layout
  - expands v to messages via a one-hot matmul against the block's 128
    rows and ACCUMULATES the gathered u into the same PSUM with an
    identity matmul; ReLU reads the PSUM directly on the ACT engine
    (v6: this removes the [P,4H] DVE add that was the DVE bottleneck)
  - segment-sums messages into the block via the one-hot scatter matmul;
    the W2 projection is applied after the segment sum (commutes)
The per-group work is software-pipelined in 3 stages (prep / z+relu /
scatter) so the in-order PE queue never waits on ACT/DVE results.
No cross-core collective is needed: each core emits its own output slice.

NOTE on PSUM: start=True arms pending-zero for the WHOLE 2KB psum bank,
so only the first matmul touching each z4 bank may set it.
"""

import numpy as np

N = 100000
D = 64
H = 128
E = 1600000
NCORES = 8
P = 128
B = 98                 # 128-node blocks per core
RANGE = B * P          # 12544 nodes owned per core
BANK = 25000           # source-bank width (int16 gather offset limit 32767)
NBANK = 4
BANKROWS = 32768       # uniform in_ap height per bank
NODESG = 3 * BANK + BANKROWS  # padded global table height
FORCE_SUBTPB = None
SINGLE_PACKET = False
REPS = 1               # timing aid: repeat the whole block loop on-device
NQ = 4                 # SWDGE queues for gather descriptor generation
IDADD = True           # accumulate u into z PSUM via PE identity matmul
RTS_ACT = True         # rt PSUM -> SBUF copy on ACT (else DVE)


def _build_program(subtpb, mode="full", single_packet=None, reps=None):
    """Build the Bass/Tile program. subtpb[k] = tiles per source bank."""
    import concourse.bass as bass
    import concourse.bacc as bacc
    import concourse.mybir as mybir
    import concourse.tile as tile

    sp = SINGLE_PACKET if single_packet is None else single_packet
    nreps = REPS if reps is None else reps
    do_gather = mode in ("full", "gatheronly")
    do_compute = mode in ("full", "nogather")

    f32 = mybir.dt.float32
    bf16 = mybir.dt.bfloat16
    i16 = mybir.dt.int16
    TILES = sum(subtpb)
    G4 = TILES // 4
    assert TILES % 4 == 0
    offs = np.concatenate([[0], np.cumsum(subtpb)]).astype(int)

    nc = bacc.Bacc(None, target_bir_lowering=False, num_swdge_queues=NQ)

    utabd = nc.dram_tensor("utab", [NODESG, P], bf16, kind="ExternalInput")
    vtabd = nc.dram_tensor("vtab", [RANGE, H], bf16, kind="ExternalInput")
    srcidx = nc.dram_tensor("srcidx16", [B // 2, P, 2 * TILES * 8], i16, kind="ExternalInput")
    dstlc = nc.dram_tensor("dstlc", [B, P, TILES], bf16, kind="ExternalInput")
    dstlcT = nc.dram_tensor("dstlcT", [B, 1, TILES * P], bf16, kind="ExternalInput")
    based = nc.dram_tensor("base", [RANGE, D], f32, kind="ExternalInput")
    w2d = nc.dram_tensor("w2b", [H, D], bf16, kind="ExternalInput")
    iotard = nc.dram_tensor("iotar4", [P, 4 * P], bf16, kind="ExternalInput")
    iotapd = nc.dram_tensor("iotap", [P, 1], f32, kind="ExternalInput")
    onesd = nc.dram_tensor("ones1", [1, P], bf16, kind="ExternalInput")
    identd = nc.dram_tensor("ident", [P, P], bf16, kind="ExternalInput")
    outd = nc.dram_tensor("out", [RANGE, D], f32, kind="ExternalOutput")

    with tile.TileContext(nc) as tc:
        with (
            tc.tile_pool(name="const", bufs=1) as cp,
            tc.tile_pool(name="blk", bufs=3) as bp,
            tc.tile_pool(name="gs", bufs=3) as gp,
            tc.tile_pool(name="work", bufs=4) as wp,
            tc.tile_pool(name="ps_bc", bufs=2, space="PSUM") as ps_bc,
            tc.tile_pool(name="ps_z", bufs=2, space="PSUM") as ps_z,
            tc.tile_pool(name="ps_rt", bufs=2, space="PSUM") as ps_rt,
            tc.tile_pool(name="ps_d", bufs=1, space="PSUM") as ps_d,
        ):
            w2b = cp.tile([H, D], bf16)
            nc.sync.dma_start(out=w2b[:], in_=w2d[:])
            iotar4 = cp.tile([P, 4 * P], bf16)
            nc.sync.dma_start(out=iotar4[:], in_=iotard[:])
            iotap = cp.tile([P, 1], f32)
            nc.sync.dma_start(out=iotap[:], in_=iotapd[:])
            ones1 = cp.tile([1, P], bf16)
            nc.sync.dma_start(out=ones1[:], in_=onesd[:])
            ident = cp.tile([P, P], bf16)
            nc.sync.dma_start(out=ident[:], in_=identd[:])

            import contextlib
            rep_ctx = tc.For_i(0, nreps, 1) if nreps > 1 else contextlib.nullcontext()
            with rep_ctx:
              for sb in range(B // 2):
                  idx_s = bp.tile([P, 2 * TILES * 8], i16, tag="idxs")
                  nc.sync.dma_start(out=idx_s[:], in_=srcidx[sb])
                  gus = gp.tile([P, 2 * TILES * P], bf16, tag="gus")
                  for k in range(NBANK if do_gather else 0):
                      if subtpb[k] == 0:
                          continue
                      nc.gpsimd.dma_gather(
                          out_ap=gus[:, 2 * offs[k] * P : 2 * offs[k + 1] * P].rearrange(
                              "p (t d) -> p t d", d=P
                          ),
                          in_ap=utabd[k * BANK : k * BANK + BANKROWS, :],
                          idxs_ap=idx_s[:, 2 * offs[k] * 8 : 2 * offs[k + 1] * 8],
                          num_idxs=2 * subtpb[k] * P,
                          num_idxs_reg=2 * subtpb[k] * P,
                          elem_size=P,
                          single_packet=sp,
                          queue_num=k % NQ,
                      )
                  if not do_gather:
                      nc.vector.memset(gus[:, 0 : 2 * TILES * P], 0.0)

                  for j in range(2):
                    b = 2 * sb + j

                    def gcol(t):
                        k = int(np.searchsorted(offs, t, side="right")) - 1
                        return 2 * offs[k] * P + j * subtpb[k] * P + (t - offs[k]) * P

                    def bank_runs(g):
                        res, t0, end = [], 4 * g, 4 * g + 4
                        while t0 < end:
                            k = int(np.searchsorted(offs, t0, side="right")) - 1
                            t1 = min(end, int(offs[k + 1]))
                            res.append((t0, t1))
                            t0 = t1
                        return res

                    dlc = bp.tile([P, TILES], bf16, tag="dlc")
                    nc.sync.dma_start(out=dlc[:], in_=dstlc[b])
                    dlcT = bp.tile([1, TILES * P], bf16, tag="dlcT")
                    nc.sync.dma_start(out=dlcT[:], in_=dstlcT[b])
                    vbs = bp.tile([P, H], bf16, tag="vbs")
                    nc.sync.dma_start(out=vbs[:], in_=vtabd[b * P : (b + 1) * P, :])
                    bst = bp.tile([P, D], f32, tag="base")
                    nc.sync.dma_start(out=bst[:], in_=based[b * P : (b + 1) * P, :])

                    if do_compute:
                        rt = ps_rt.tile([H, P], f32, tag="rt")
                        s2_l, s2T_l, rl_l = {}, {}, {}
                        for st in range(G4 + 2):
                            if st < G4:
                                g = st
                                s2 = wp.tile([P, 4 * P], bf16, tag="s2")
                                nc.vector.tensor_tensor(
                                    out=s2[:].rearrange("p (t j) -> p t j", t=4),
                                    in0=dlc[:, 4 * g : 4 * g + 4].to_broadcast([P, 4, P]),
                                    in1=iotar4[:, :].rearrange("p (t j) -> p t j", t=4),
                                    op=mybir.AluOpType.is_equal,
                                )
                                bc = ps_bc.tile([P, 4 * P], f32, tag="bc")
                                nc.tensor.matmul(
                                    out=bc[:],
                                    lhsT=ones1[:],
                                    rhs=dlcT[:, 4 * g * P : 4 * (g + 1) * P],
                                    start=True,
                                    stop=True,
                                )
                                s2T = wp.tile([P, 4 * P], bf16, tag="s2T")
                                nc.vector.tensor_tensor(
                                    out=s2T[:],
                                    in0=bc[:],
                                    in1=iotap[:, :].to_broadcast([P, 4 * P]),
                                    op=mybir.AluOpType.is_equal,
                                )
                                s2_l[g], s2T_l[g] = s2, s2T
                            if 1 <= st <= G4:
                                g = st - 1
                                z4 = ps_z.tile([P, 4 * H], f32, tag="z4")
                                for t in range(4):
                                    # start=True arms pending-zero for the
                                    # WHOLE 2KB psum bank: only the first
                                    # matmul in the bank may set it.
                                    nc.tensor.matmul(
                                        out=z4[:, t * H : (t + 1) * H],
                                        lhsT=s2T_l[g][:, t * P : (t + 1) * P],
                                        rhs=vbs[:],
                                        start=(t == 0) if IDADD else True,
                                        stop=not IDADD,
                                        skip_group_check=IDADD,
                                    )
                                if IDADD:
                                    runs = bank_runs(g)
                                    for ri, (t0, t1) in enumerate(runs):
                                        nc.tensor.matmul(
                                            out=z4[
                                                :, (t0 - 4 * g) * H : (t1 - 4 * g) * H
                                            ],
                                            lhsT=ident[:],
                                            rhs=gus[:, gcol(t0) : gcol(t0) + (t1 - t0) * P],
                                            start=False,
                                            stop=(ri == len(runs) - 1),
                                            skip_group_check=True,
                                        )
                                    rl = wp.tile([P, 4 * H], bf16, tag="rl")
                                    nc.scalar.activation(
                                        out=rl[:],
                                        in_=z4[:],
                                        func=mybir.ActivationFunctionType.Relu,
                                    )
                                else:
                                    zb = wp.tile([P, 4 * H], bf16, tag="zb")
                                    for (t0, t1) in bank_runs(g):
                                        nc.vector.tensor_tensor(
                                            out=zb[:, (t0 - 4 * g) * H : (t1 - 4 * g) * H],
                                            in0=z4[:, (t0 - 4 * g) * H : (t1 - 4 * g) * H],
                                            in1=gus[:, gcol(t0) : gcol(t0) + (t1 - t0) * P],
                                            op=mybir.AluOpType.add,
                                        )
                                    rl = wp.tile([P, 4 * H], bf16, tag="rl")
                                    nc.scalar.activation(
                                        out=rl[:],
                                        in_=zb[:],
                                        func=mybir.ActivationFunctionType.Relu,
                                    )
                                rl_l[g] = rl
                            if st >= 2:
                                g = st - 2
                                for t in range(4):
                                    nc.tensor.matmul(
                                        out=rt[:],
                                        lhsT=rl_l[g][:, t * H : (t + 1) * H],
                                        rhs=s2_l[g][:, t * P : (t + 1) * P],
                                        start=(g == 0 and t == 0),
                                        stop=(g == G4 - 1 and t == 3),
                                    )
                        rts = wp.tile([H, P], bf16, tag="rts")
                        if RTS_ACT:
                            nc.scalar.copy(out=rts[:], in_=rt[:])
                        else:
                            nc.vector.tensor_copy(out=rts[:], in_=rt[:])
                        delta = ps_d.tile([P, D], f32, tag="delta")
                        nc.tensor.matmul(
                            out=delta[:], lhsT=rts[:], rhs=w2b[:], start=True, stop=True
                        )
                    osb = bp.tile([P, D], f32, tag="osb")
                    if do_compute:
                        nc.vector.tensor_add(out=osb[:], in0=bst[:], in1=delta[:])
                    elif do_gather:
                        nc.vector.tensor_add(out=osb[:], in0=bst[:], in1=gus[:, 0:D])
                    else:
                        nc.vector.tensor_copy(out=osb[:], in_=bst[:])
                    nc.sync.dma_start(out=outd[b * P : (b + 1) * P, :], in_=osb[:])

    nc.compile()
    return nc


def _wrap16(a):
    """Pack a flat int array -> [128, len/16] int16 in the dma_gather idx
    layout (idx q at [q%16, q//16], replicated across the 8 gpsimd cores)."""
    a = np.asarray(a, np.int16).reshape(-1, 16).T  # [16, n/16]
    return np.tile(a, (8, 1))  # [128, n/16]


def _prep(nodes, edges, W1, b1, W2, b2):
    """Host-side: sort directed messages by destination, shard by dest range,
    group by 128-node dest block and source bank. Returns (in_maps, subtpb)."""
    import ml_dtypes

    def tobf(x):
        return np.ascontiguousarray(x).astype(ml_dtypes.bfloat16)

    nodes = np.ascontiguousarray(nodes, dtype=np.float32)
    edges = np.asarray(edges)
    dst = np.concatenate([edges[:, 0], edges[:, 1]]).astype(np.int64)
    src = np.concatenate([edges[:, 1], edges[:, 0]]).astype(np.int64)
    # sort by (dest block-of-128, source bank): dest-block-major key
    sbank_all = np.minimum(src // BANK, NBANK - 1)
    key = ((dst >> 7) << 2) | sbank_all
    order = np.argsort(key, kind="stable")
    dst = dst[order]
    src = src[order]
    sbank = sbank_all[order]

    bounds = np.searchsorted(dst, np.arange(NCORES + 1) * RANGE)
    per_core = []
    cnts = np.zeros((NCORES, B, NBANK), np.int64)
    for c in range(NCORES):
        dl = dst[bounds[c] : bounds[c + 1]] - c * RANGE
        sl = src[bounds[c] : bounds[c + 1]]
        sb = sbank[bounds[c] : bounds[c + 1]]
        blk = dl >> 7
        np.add.at(cnts[c], (blk, sb), 1)
        per_core.append((dl, sl, sb, blk))

    maxk = cnts.max(axis=(0, 1))            # per-bank max count
    subtpb = [int(-(-m // P)) for m in maxk]
    subtpb = [max(s, 1) for s in subtpb]
    while sum(subtpb) % 4:
        subtpb[0] += 1
    if FORCE_SUBTPB is not None:
        subtpb = list(FORCE_SUBTPB)
    TILES = sum(subtpb)
    offs = np.concatenate([[0], np.cumsum(subtpb)]).astype(np.int64)

    nodes_my = np.zeros((max(NCORES * RANGE, N) + RANGE, D), np.float32)
    nodes_my[:N] = nodes

    iotar4 = np.ascontiguousarray(
        np.broadcast_to(
            np.tile(np.arange(P, dtype=np.float32), 4)[None, :], (P, 4 * P)
        )
    )
    iotap = np.arange(P, dtype=np.float32).reshape(P, 1)
    ones1 = np.ones((1, P), np.float32)
    ident = np.eye(P, dtype=np.float32)
    W1 = np.ascontiguousarray(W1, dtype=np.float32)
    W2 = np.ascontiguousarray(W2, dtype=np.float32)

    # host basis change: u = h @ W1b (gathered), v = h @ W1a + b1 (per block)
    u_all = nodes @ W1[D:]                       # [N, H] f32
    utab = np.zeros((NODESG, P), np.float32)
    utab[:N] = u_all
    utab_bf = tobf(utab)

    in_maps = []
    for c in range(NCORES):
        dl, sl, sb, blk = per_core[c]
        # slot of each message: position within its (block, bank) group
        grp = blk * NBANK + sb
        gstarts = np.concatenate(
            [[0], np.cumsum(np.bincount(grp, minlength=B * NBANK))]
        )[:-1]
        m = np.arange(len(dl)) - gstarts[grp]
        slot = (offs[sb] * P + m).astype(np.int64)  # slot within block slotspace
        tt = slot // P
        pp = slot % P

        src_flat = np.zeros((B, TILES * P), np.int64)
        dlc_arr = np.full((B, P, TILES), -1.0, np.float32)
        dlcT_arr = np.full((B, 1, TILES * P), -1.0, np.float32)
        src_flat[blk, slot] = sl - sb * BANK
        dlc_arr[blk, pp, tt] = (dl & 127).astype(np.float32)
        dlcT_arr[blk, 0, slot] = (dl & 127).astype(np.float32)

        srcidx16 = np.zeros((B // 2, P, 2 * TILES * 8), np.int16)
        for s in range(B // 2):
            parts = []
            for k in range(NBANK):
                for j in range(2):
                    parts.append(
                        _wrap16(
                            src_flat[2 * s + j, offs[k] * P : offs[k + 1] * P]
                        )
                    )
            srcidx16[s] = np.concatenate(parts, axis=1)

        deg = np.bincount(dl, minlength=RANGE).astype(np.float32)
        mynodes = np.ascontiguousarray(nodes_my[c * RANGE : (c + 1) * RANGE])
        base = mynodes + deg[:, None] * b2[None, :].astype(np.float32)
        vtab = mynodes @ W1[:D] + b1.astype(np.float32)[None, :]

        in_maps.append(
            {
                "utab": utab_bf,
                "vtab": tobf(vtab),
                "srcidx16": srcidx16,
                "dstlc": tobf(dlc_arr),
                "dstlcT": tobf(dlcT_arr),
                "base": np.ascontiguousarray(base),
                "w2b": tobf(W2),
                "iotar4": tobf(iotar4),
                "iotap": iotap,
                "ones1": tobf(ones1),
                "ident": tobf(ident),
            }
        )
    return in_maps, subtpb


def kernel(nodes, edges, W1, b1, W2, b2):
    from concourse.bass_utils import run_bass_kernel_spmd

    in_maps, subtpb = _prep(nodes, edges, W1, b1, W2, b2)
    nc = _build_program(subtpb)
    res = run_bass_kernel_spmd(nc, in_maps, list(range(NCORES)))
    outs = [np.asarray(r["out"]) for r in res.results]
    return np.concatenate(outs, axis=0)[:N]
